# revision 1
# baseline (speedup 1.0000x reference)
"""Trainium2 Bass kernel for nn_BDHGPURefStabilized.

Model (per batch element b, scan over T steps):
    v_t   = token_emb[tok_t]                         # [D]
    xt    = 0.97*x + v_t @ Dx.T                      # [N]
    xt    = xt / (sum|xt| + 1e-6)
    xt    = where(xt > 0.02*max(xt), xt, 0)
    a*    = rho @ xt                                 # fast-weight read [D]
    y     = LN(a*) @ Dy.T                            # [N]
    yt    = relu(y) * relu(xt)
    v*_t  = LN(yt @ E.T)                             # output row [D]
    rho   = 0.97*(rho + v_t (x) xt)                  # rank-1 fast-weight update

Kernel strategy (8 NeuronCores, data-parallel over batch B=8, one batch
element per core, zero collectives):

 - The scan's serial spine (xt recurrence) runs in fp32 exactly like the
   reference, so the sparsifying threshold pattern matches bit-for-bit in
   practice.
 - rho is never materialized.  With the rescaling rho_t = 0.97^t * rho_hat_t,
   rho_hat is a pure (decay-free) sum of rank-1 terms, so
       a*_t = 0.97^t * sum_{s<t} (0.97^-s v_s) * (xt_s . xt_t)
   i.e. an attention read against the stored xt history and pre-scaled
   embedding rows.  The 0.97^t factor is folded exactly into the following
   LayerNorm by adjusting its epsilon (LN is scale-invariant up to eps).
 - Output-path matmuls run in fp16 (PSUM accumulates fp32); the xt spine
   stays fp32.
 - Layout: n = c*128 + j with j on partitions; xt lives as [128, 16].
   Per-d-vectors (a*, LN(a*), u, v*) live as rows [1, 128].
 - token gather, layout transposes of the static weights and the decay
   constants are prepared host-side (pure indexing / casting); all model
   FLOPs (including v @ Dx.T for all steps) run on device.

Output per core: [128(d), T] fp32 columns; host reassembles [B, T, D].
"""

import math
from contextlib import ExitStack

import numpy as np

import concourse.bass as bass
import concourse.bacc as bacc
import concourse.tile as tile
from concourse import mybir

F32 = mybir.dt.float32
F16 = mybir.dt.float16
AX = mybir.AxisListType
OP = mybir.AluOpType
AF = mybir.ActivationFunctionType

N, D, V = 2048, 128, 131072
C = N // 128  # 16 column-chunks of n; n = c*128 + j
U_DECAY, X_DECAY, THR = 0.97, 0.97, 0.02


def scan_program(tc, outs, ins, T):
    """Emit the full per-core scan as a Tile program.

    ins:  dict of DRAM APs: DxT[128,2048]f32, DyTr[128,16,128]f16,
          ETr[128,16,128]f16, Vt[128,T]f32, Vh[128,SC,128]f16,
          ones_row[1,128]f32, idn128[128,128]f32, idn1_32[1,1]f32,
          idn1_16[1,1]f16
    outs: dict with out[128,T]f32
    """
    nc = tc.nc
    ctx = ExitStack()
    SC = (T + 127) // 128  # history chunks along s
    W32 = N + T + 512      # packed f32 input width
    W16 = 2 * N + SC * 128 + 128  # packed f16 input width

    with ctx:
        wpool = ctx.enter_context(tc.tile_pool(name="weights", bufs=1))
        spool = ctx.enter_context(tc.tile_pool(name="step", bufs=3))
        xpool = ctx.enter_context(tc.tile_pool(name="xstate", bufs=3))
        scal = ctx.enter_context(tc.tile_pool(name="scal", bufs=4))

        # ---- load packed inputs (2 DMAs, so downstream waits stay small) ----
        B32 = wpool.tile([128, W32], F32, tag="B32")
        B16 = wpool.tile([128, W16], F16, tag="B16")
        # single SWDGE queue: the consumers then wait on one DMA semaphore
        # instead of one per HWDGE queue the transfer would be split across.
        nc.gpsimd.dma_start(out=B32, in_=ins["B32"])
        nc.gpsimd.dma_start(out=B16, in_=ins["B16"])
        DxT = B32[:, 0:N]
        Vt = B32[:, N:N + T]
        idn128 = B32[:, N + T:N + T + 128]
        ones_row = B32[0:1, N + T + 128:N + T + 256]
        ones_col = B32[:, N + T + 128:N + T + 129]
        idn1_32 = B32[0:1, N + T + 128:N + T + 129]
        row097 = B32[0:1, N + T + 256:N + T + 384]
        row002 = B32[0:1, N + T + 384:N + T + 512]
        DyTr = B16[:, 0:N].rearrange("p (c j) -> p c j", c=C)
        ETr = B16[:, N:2 * N].rearrange("p (c j) -> p c j", c=C)
        Vh = B16[:, 2 * N:2 * N + SC * 128].rearrange("p (s j) -> p s j", s=SC)
        idn1_16 = B16[0:1, 2 * N + SC * 128:2 * N + SC * 128 + 1]

        # persistent SBUF state
        P_sb = wpool.tile([128, C, T], F32, tag="P_sb")      # v_t @ Dx.T, all steps
        Xh = wpool.tile([128, C, T], F16, tag="Xh")          # xt history
        out_cols = wpool.tile([128, T], F32, tag="out_cols")  # v*_t columns
        nc.vector.memset(out_cols, 0.0)

        # ---- P_all = Dx @ V  (device-side, fp32) ----
        with tc.tile_pool(name="psetup", bufs=2, space="PSUM") as psetup:
            for c in range(C):
                p_ps = psetup.tile([128, T], F32, tag="pp")
                nc.tensor.matmul(
                    p_ps, DxT[:, c * 128:(c + 1) * 128], Vt,
                    start=True, stop=True,
                )
                nc.scalar.copy(P_sb[:, c, :], p_ps)

        # PSUM pools for the steady-state loop (8 banks total)
        pg = ctx.enter_context(tc.tile_pool(name="pg", bufs=2, space="PSUM"))
        pgt = ctx.enter_context(tc.tile_pool(name="pgt", bufs=1, space="PSUM"))
        pa = ctx.enter_context(tc.tile_pool(name="pa", bufs=1, space="PSUM"))
        ptp = ctx.enter_context(tc.tile_pool(name="ptp", bufs=1, space="PSUM"))
        py = ctx.enter_context(tc.tile_pool(name="py", bufs=1, space="PSUM"))
        pu = ctx.enter_context(tc.tile_pool(name="pu", bufs=1, space="PSUM"))
        pvt = ctx.enter_context(tc.tile_pool(name="pvt", bufs=1, space="PSUM"))

        def emit_spine(t, prev):
            # xtp = 0.97*xt_{t-1} + P[:, :, t]; xt kept unnormalized (xtu)
            # with its 1/s factor folded into the scalar (tpr col 133).
            # The whole spine is the serial recurrence: emit it at high
            # priority so its ops sit ahead of chain matmuls in the in-order
            # engine streams.
            with tc.high_priority(offset=260):
                xtp = spool.tile([128, C], F32, tag="xtp")
                if t == 0:
                    nc.vector.tensor_copy(xtp, P_sb[:, :, 0])
                else:
                    xtu_prev, tpr_prev = prev
                    nc.vector.scalar_tensor_tensor(
                        out=xtp, in0=xtu_prev, scalar=tpr_prev[:, 133:134],
                        in1=P_sb[:, :, t], op0=OP.mult, op1=OP.add,
                    )
                # partials per partition: [:,0]=sum|.|, [:,1]=max
                part2 = spool.tile([128, 2], F32, tag="part2")
                nc.vector.tensor_reduce(
                    out=part2[:, 0:1], in_=xtp, axis=AX.X, op=OP.add,
                    apply_absolute_value=True)
                nc.vector.tensor_reduce(
                    out=part2[:, 1:2], in_=xtp, axis=AX.X, op=OP.max)
                # cross-partition: max via transpose+reduce, sum via
                # ones-matmul; tpr also holds the broadcast columns:
                # [0:1,0:128]=maxT, [0:1,128:129]=s, 132=1/s, 133=0.97/s,
                # 134=0.02*m.
                tpr = ptp.tile([128, 136], F32, tag="tpr")
                nc.tensor.transpose(tpr[0:1, 0:128], part2[:, 1:2], idn128)
                nc.tensor.matmul(tpr[0:1, 128:129], part2[:, 0:1], ones_col,
                                 start=True, stop=True)
                s1 = scal.tile([1, 4], F32, tag="s1")
                nc.vector.tensor_reduce(
                    out=s1[:, 1:2], in_=tpr[0:1, 0:128], axis=AX.X, op=OP.max)
                # 1/(s+1e-6) ~= 1/s (relative error ~1e-8; the mask is
                # computed pre-normalization so this cannot flip it)
                nc.vector.reciprocal(out=s1[:, 2:3], in_=tpr[0:1, 128:129])
                # broadcasts: 132 = 1/s', 133 = 0.97/s', 134 = 0.02*m
                nc.tensor.matmul(tpr[:, 132:133], ones_row, s1[:, 2:3],
                                 start=True, stop=True)
                nc.tensor.matmul(tpr[:, 133:134], row097, s1[:, 2:3],
                                 start=True, stop=True)
                nc.tensor.matmul(tpr[:, 134:135], row002, s1[:, 1:2],
                                 start=True, stop=True)
                # xtu = (xtp > thr) * xtp   (unnormalized, masked)
                xtu = xpool.tile([128, C], F32, tag="xtu")
                nc.vector.scalar_tensor_tensor(
                    out=xtu, in0=xtp, scalar=tpr[:, 134:135], in1=xtp,
                    op0=OP.is_gt, op1=OP.mult,
                )
                rt_sb = spool.tile([128, 2], F32, tag="rt_sb")
                nc.vector.tensor_copy(rt_sb, tpr[:, 132:134])
                # normalized history append + relu(xt), both on gpsimd
                nc.gpsimd.tensor_scalar(
                    out=Xh[:, :, t], in0=xtu, scalar1=rt_sb[:, 0:1],
                    scalar2=None, op0=OP.mult)
                w16 = spool.tile([128, C], F16, tag="w16")
                nc.gpsimd.tensor_scalar(
                    out=w16, in0=xtu, scalar1=rt_sb[:, 0:1], scalar2=0.0,
                    op0=OP.mult, op1=OP.max)
            return xtu, tpr, w16

        def emit_chain(t, w16):
            # output chain for step t (t >= 1); lags the spine by one step.
            t1 = min(t, 128)
            t2 = t - t1
            g_ps = pg.tile([1, T], F32, tag="g")
            for c in range(C):
                nc.tensor.matmul(
                    g_ps[:, 0:t], Xh[:, c, t:t + 1], Xh[:, c, 0:t],
                    start=(c == 0), stop=(c == C - 1),
                )
            g16 = spool.tile([1, T], F16, tag="g16")
            nc.scalar.copy(g16[:, 0:t], g_ps[:, 0:t])
            gt_ps = pgt.tile([128, 4], F16, tag="gt")
            gS = spool.tile([128, 4], F16, tag="gS")
            nc.tensor.transpose(gt_ps[0:t1, 0:1], g16[:, 0:t1], idn1_16)
            nc.scalar.copy(gS[0:t1, 0:1], gt_ps[0:t1, 0:1])
            if t2 > 0:
                nc.tensor.transpose(gt_ps[0:t2, 2:3], g16[:, 128:t], idn1_16)
                nc.scalar.copy(gS[0:t2, 2:3], gt_ps[0:t2, 2:3])
            a_ps = pa.tile([1, 128], F32, tag="a")
            nc.tensor.matmul(a_ps, gS[0:t1, 0:1], Vh[0:t1, 0, :],
                             start=True, stop=(t2 == 0))
            if t2 > 0:
                nc.tensor.matmul(a_ps, gS[0:t2, 2:3], Vh[0:t2, 1, :],
                                 start=False, stop=True)

            # evacuate PSUM row immediately (frees the single-buffered bank
            # early, shortening the pipeline initiation interval)
            a_sb = spool.tile([1, 128], F32, tag="a_sb")
            nc.scalar.copy(a_sb, a_ps)
            # LN(a*) (scaled: eps_t = 1e-6 * 0.97^-t)
            lnA16 = _layernorm_row(
                tc, spool, scal, a_sb, 1e-6 * (U_DECAY ** (-t)), F16)
            lnT = pvt.tile([128, 1], F16, tag="colT")
            nc.tensor.transpose(lnT, lnA16, idn1_16)
            lnA16c = spool.tile([128, 1], F16, tag="lnA16c")
            nc.scalar.copy(lnA16c, lnT)

            # y = LN(a*) @ Dy.T (column form: Dy chunks as stationary weights)
            y_ps = py.tile([128, C], F32, tag="y")
            for c in range(C):
                nc.tensor.matmul(
                    y_ps[:, c:c + 1], DyTr[:, c, :], lnA16c,
                    start=True, stop=True,
                )
            # yt = relu(y) * relu(xt)   (fp16; ACT relu, gpsimd multiply)
            ry16 = spool.tile([128, C], F16, tag="ry16")
            nc.scalar.activation(out=ry16, in_=y_ps, func=AF.Relu)
            yt16 = spool.tile([128, C], F16, tag="yt16")
            nc.gpsimd.tensor_tensor(out=yt16, in0=ry16, in1=w16, op=OP.mult)

            # u = E @ yt
            u_ps = pu.tile([1, 128], F32, tag="u")
            for c in range(C):
                nc.tensor.matmul(
                    u_ps, yt16[:, c:c + 1], ETr[:, c, :],
                    start=(c == 0), stop=(c == C - 1),
                )

            u_sb = spool.tile([1, 128], F32, tag="u_sb")
            nc.scalar.copy(u_sb, u_ps)
            # v* = LN(u), store column
            vst = _layernorm_row(tc, spool, scal, u_sb, 1e-6, F32)
            vT = pvt.tile([128, 1], F32, tag="colT")
            nc.tensor.transpose(vT, vst, idn1_32)
            nc.scalar.copy(out_cols[:, t:t + 1], vT)

        # software-pipelined emission: spine runs one step ahead of the
        # output chain, so PE/DVE/ACT work of adjacent steps can overlap.
        prev = None
        saved_w16 = {}
        for t in range(T):
            xtu, tpr, w16 = emit_spine(t, prev)
            prev = (xtu, tpr)
            saved_w16[t] = w16
            if t - 1 >= 1:
                emit_chain(t - 1, saved_w16.pop(t - 1))
        if T - 1 >= 1:
            emit_chain(T - 1, saved_w16.pop(T - 1))

        nc.sync.dma_start(out=outs["out"], in_=out_cols)


def _layernorm_row(tc, spool, scal, row_ps, eps, out_dtype):
    """LayerNorm over a [1, 128] PSUM row, torch-style (ddof=1, eps on std).

    Returns a [1, 128] SBUF tile of out_dtype.
    """
    nc = tc.nc
    stats = scal.tile([1, 6], F32, tag="ln_stats")
    mv = scal.tile([1, 2], F32, tag="ln_mv")
    nc.vector.bn_stats(out=stats, in_=row_ps)
    nc.vector.bn_aggr(out=mv, in_=stats)
    sd = scal.tile([1, 2], F32, tag="ln_sd")
    # sd[0] = sqrt(var * 128/127)  (unbiased std)
    nc.scalar.activation(
        out=sd[:, 0:1], in_=mv[:, 1:2], func=AF.Sqrt, scale=float(D) / (D - 1))
    nc.vector.tensor_scalar(
        out=sd[:, 1:2], in0=sd[:, 0:1], scalar1=float(eps), scalar2=None,
        op0=OP.add)
    rstd = scal.tile([1, 2], F32, tag="ln_rstd")
    nc.vector.reciprocal(out=rstd[:, 0:1], in_=sd[:, 1:2])
    out = spool.tile([1, 128], out_dtype, tag=f"ln_out_{out_dtype}")
    # out = (row - mean) * rstd
    nc.vector.tensor_scalar(
        out=out, in0=row_ps, scalar1=mv[:, 0:1], scalar2=rstd[:, 0:1],
        op0=OP.subtract, op1=OP.mult)
    return out


# ----------------------------------------------------------------------------
# host side
# ----------------------------------------------------------------------------

def _host_prep_shared(E, Dx, Dy, T):
    """Packed B32/B16 templates (per-core slots for Vt/Vh left zero)."""
    SC = (T + 127) // 128
    W32 = N + T + 512
    W16 = 2 * N + SC * 128 + 128
    B32 = np.zeros((128, W32), dtype=np.float32)
    B32[:, 0:N] = Dx.T
    B32[:, N + T:N + T + 128] = np.eye(128, dtype=np.float32)
    B32[:, N + T + 128:N + T + 256] = 1.0
    B32[:, N + T + 256:N + T + 384] = X_DECAY
    B32[:, N + T + 384:N + T + 512] = THR
    B16 = np.zeros((128, W16), dtype=np.float16)
    B16[:, 0:N] = Dy.reshape(C, 128, D).transpose(2, 0, 1).reshape(128, N)  # [d,(c,j)]
    B16[:, N:2 * N] = E.reshape(D, C, 128).transpose(2, 1, 0).reshape(128, N)  # [j,(c,d)]
    B16[:, 2 * N + SC * 128:] = 1.0
    return B32, B16


def _host_prep_core(B32t, B16t, token_emb, tokens_b, T):
    SC = (T + 127) // 128
    B32 = B32t.copy()
    B16 = B16t.copy()
    V_all = token_emb[tokens_b].astype(np.float32)         # [T, 128] host gather
    B32[:, N:N + T] = V_all.T
    decay = (U_DECAY ** (-np.arange(T, dtype=np.float64))).astype(np.float32)
    Vh_flat = np.zeros((SC * 128, 128), dtype=np.float32)
    Vh_flat[:T] = V_all * decay[:, None]
    B16[:, 2 * N:2 * N + SC * 128] = (
        Vh_flat.reshape(SC, 128, 128).transpose(1, 0, 2).reshape(128, SC * 128))
    return dict(B32=B32, B16=B16)


_PROGRAM_CACHE = {}
RUN_KWARGS = {}      # extra kwargs forwarded to run_bass_kernel_spmd (e.g. trace)
LAST_RESULTS = None  # BassKernelResults of the most recent kernel() call


def _build(T):
    key = T
    if key in _PROGRAM_CACHE:
        return _PROGRAM_CACHE[key]
    SC = (T + 127) // 128
    W32 = N + T + 512
    W16 = 2 * N + SC * 128 + 128
    nc = bacc.Bacc("TRN2")
    ins = {
        "B32": nc.dram_tensor("B32", [128, W32], F32, kind="ExternalInput").ap(),
        "B16": nc.dram_tensor("B16", [128, W16], F16, kind="ExternalInput").ap(),
    }
    outs = {
        "out": nc.dram_tensor("out", [128, T], F32, kind="ExternalOutput").ap(),
    }
    with tile.TileContext(nc) as tc:
        scan_program(tc, outs, ins, T)
    nc.compile()  # bacc lowering: splits multi-waits to the 1-slot HW limit
    _PROGRAM_CACHE[key] = (nc, ins, outs)
    return _PROGRAM_CACHE[key]


def kernel(E, Dx, Dy, token_emb, tokens):
    from concourse.bass_utils import run_bass_kernel_spmd

    E = np.asarray(E, dtype=np.float32)
    Dx = np.asarray(Dx, dtype=np.float32)
    Dy = np.asarray(Dy, dtype=np.float32)
    token_emb = np.asarray(token_emb, dtype=np.float32)
    tokens = np.asarray(tokens)
    B, T = tokens.shape

    nc, ins, outs = _build(T)
    B32t, B16t = _host_prep_shared(E, Dx, Dy, T)
    in_maps = [
        _host_prep_core(B32t, B16t, token_emb, tokens[b], T) for b in range(B)
    ]

    res = run_bass_kernel_spmd(nc, in_maps, core_ids=list(range(B)), **RUN_KWARGS)
    global LAST_RESULTS
    LAST_RESULTS = res
    out = np.stack([r["out"].T for r in res.results])  # [B, T, 128]
    return out.astype(np.float32)

